# revision 26
# baseline (speedup 1.0000x reference)
"""2D DWT (db2, FFT-equivalent circular conv) as TensorE matmuls on 8 trn2 cores.

Math: for each (b,c) slice X (128x128), with F[k,j] = w[t] at k=(2j+2-t) mod 128
(the circular 4-tap filter + stride-2 decimation as a 128x64 matrix):
    LL = Fl^T X Fl,  LH = Fh^T X Fl,  HL = Fl^T X Fh,  HH = Fh^T X Fh.
With W2 = [Fl | Fh] (128x128):
    stage 1:  out1 = X^T @ W2 = [B_lT | B_hT]           (w on partitions)
    stage 2:  out2 = W2^T @ out1 = [[LL^T, LH^T], [HL^T, HH^T]]
out2 has partitions = j (W-direction output), free = i (H-direction output);
the final transpose of each 64x64 quadrant happens on the host at gather time.

Everything rides fp16 end to end: x, W2, the stage-1 intermediate, and the
DRAM output are all fp16 (PSUM accumulates in fp32).  Input quantization to
fp16 bounds the output error at ~1e-3 relative to max, far inside the 2e-2
gate, and it halves DMA traffic (the kernel is HBM-bound) while tripling
TensorE throughput vs an fp16 hi+lo split.  X is pre-scaled by 1024 on the
host so near-zero values stay out of fp16 subnormal range; the scale is
divided out in the stage-2 output copy (free affine on ACT / mul on DVE).

The whole per-core problem fits in SBUF (x/y/out are 24.6 KiB/partition
each), so x, the stage-1 intermediate y, and the staged output live in
single whole-shard tiles.  All DMA triggers ride the sync ring: inputs
back-to-back first, outputs strictly after, so no output trigger's
semaphore wait ever blocks an input and the ACT/DVE queues carry only
compute.  Work is software-pipelined per bank with stage 2 lagging
stage 1 by one bank on TensorE; banks are graduated (4-slice at both
ends, 8-slice in the middle) so the first drain chain -- and with it the
DVE and output streams -- starts ~2us sooner and the final drain tail is
short.  Each PSUM->SBUF drain is one wide instruction, and each stage
owns one engine (stage 1 -> ACT, stage 2 -> DVE) so neither engine's
in-order queue ever holds the other stage's work hostage to a semaphore.
Dummy matmuls on a memset tile warm the PE through the HAM half-clock
power ramp while the first input chunks are still in flight.

Sharding: 768 (b,c) slices split contiguously, 96 per core; pure data parallel.
Per-core input shards are transposed on the host to (h, s, w) so every DMA
reads multi-KB contiguous runs per partition.
"""

import numpy as np

_NCORES = 8
_S = 96          # slices per core
_N = 128
_SCALE = 1024.0  # fp16 subnormal guard; divided out in stage-2 copies

_compiled = None


def _build_w2(w_l: np.ndarray, w_h: np.ndarray) -> np.ndarray:
    W2 = np.zeros((_N, _N), dtype=np.float32)
    for col, w in ((0, w_l), (64, w_h)):
        w = np.asarray(w, dtype=np.float32).reshape(-1)
        L = w.shape[0]
        for j in range(_N // 2):
            for t in range(L):
                W2[(2 * j + L // 2 - t) % _N, col + j] += w[t]
    return W2


def _build_nc():
    import concourse.bacc as bacc
    import concourse.tile as tile
    import concourse.mybir as mybir

    f16 = mybir.dt.float16
    f32 = mybir.dt.float32
    nc = bacc.Bacc("TRN2", target_bir_lowering=False, debug=False)

    xh = nc.dram_tensor("xh", [_N, _S, _N], f16, kind="ExternalInput")  # (h, s, w)
    w2d = nc.dram_tensor("w2", [_N, _N], f16, kind="ExternalInput")
    out_t = nc.dram_tensor("out_t", [_N, _S, _N], f16, kind="ExternalOutput")
    inv = 1.0 / _SCALE

    # few input triggers: each costs ~650ns of sync-sequencer descriptor
    # generation, and the per-bank output triggers queue behind all of them
    in_chunks = [4, 4, 8, 16, 32, 32]
    assert sum(in_chunks) == _S
    # graduated superbanks: 4-slice banks at both ends shorten the first
    # drain chain (earlier DVE/output streams) and the final drain tail
    banks = [4, 4] + [8] * 10 + [4, 4]
    assert sum(banks) == _S
    NB = len(banks)
    s0 = [0]
    for bs in banks:
        s0.append(s0[-1] + bs)
    LAG = 1                                    # s2(b) issues after s1(b+LAG)
    copy_fn = mybir.ActivationFunctionType.Copy
    with tile.TileContext(nc) as tc:
        with (
            tc.tile_pool(name="singles", bufs=1) as singles,
            tc.tile_pool(name="ps1", bufs=2, space="PSUM") as ps1p,
            tc.tile_pool(name="ps2", bufs=2, space="PSUM") as ps2p,
        ):
            w2_sb = singles.tile([_N, _N], f16)
            # weights ride the scalar ring: its sequencer is free at body
            # start, so the tiny w2 descriptor generation runs concurrently
            # with chunk 0's on the sync ring instead of ahead of it
            nc.scalar.dma_start(out=w2_sb[:], in_=w2d[:])
            x_sb = singles.tile([_N, _S * _N], f16)
            y_sb = singles.tile([_N, _S * _N], f16)
            o_sb = singles.tile([_N, _S * _N], f16)

            # input stream: back-to-back on the sync ring
            c0 = 0
            for G in in_chunks:
                nc.sync.dma_start(
                    out=x_sb[:, c0 * _N : (c0 + G) * _N].rearrange(
                        "p (s w) -> p s w", s=G
                    ),
                    in_=xh[:, c0 : c0 + G, :],
                )
                c0 += G

            # warm the PE while the first input chunk is in flight: the HAM
            # power ramp holds the PE at half clock early on, so burn the
            # DMA-wait window on dummy matmuls.  The operand is an on-chip
            # memset tile, so the warm-up has no DMA dependency at all and
            # the PE stream starts the moment the body begins.
            warm_w = singles.tile([_N, _N], f16)
            nc.gpsimd.memset(warm_w[:], 0.0)
            warm = ps1p.tile([_N, 1024], f32, tag="ps1")
            for k in range(13):
                nc.tensor.matmul(
                    warm[:, :_N], lhsT=warm_w[:], rhs=warm_w[:],
                    start=(k == 0), stop=(k == 12),
                )

            for t in range(NB + LAG):
                if t < NB:
                    # stage 1, bank t -> PSUM, drained by one ACT copy
                    bs = banks[t]
                    ps1 = ps1p.tile([_N, 1024], f32, tag="ps1")
                    for k in range(bs):
                        s = s0[t] + k
                        nc.tensor.matmul(
                            ps1[:, k * _N : (k + 1) * _N],
                            lhsT=x_sb[:, s * _N : (s + 1) * _N],
                            rhs=w2_sb[:],
                            start=True,
                            stop=True,
                        )
                    # stage-1 drains always ride ACT: a single-purpose FIFO
                    # never holds a stage-2 copy hostage to a stage-1 sem
                    nc.scalar.copy(
                        out=y_sb[:, s0[t] * _N : s0[t + 1] * _N],
                        in_=ps1[:, : bs * _N],
                    )

                if t >= LAG:
                    # stage 2, bank b: 512-col matmuls into one PSUM tile,
                    # drained by a single DVE copy (the pacing engine)
                    b = t - LAG
                    bs = banks[b]
                    g0 = s0[b] * _N
                    ps2 = ps2p.tile([_N, 1024], f32, tag="ps2")
                    for h in range((bs * _N + 511) // 512):
                        gw = min(512, bs * _N - h * 512)
                        nc.tensor.matmul(
                            ps2[:, h * 512 : h * 512 + gw],
                            lhsT=w2_sb[:], rhs=y_sb[:, g0 + h * 512 : g0 + h * 512 + gw],
                            start=True, stop=True,
                        )
                    nc.vector.tensor_scalar_mul(
                        o_sb[:, g0 : g0 + bs * _N], ps2[:, : bs * _N], inv
                    )

                    # per-bank output DMA: the stream starts as soon as the
                    # very first bank's drain lands.  The second-to-last
                    # trigger rides the scalar ring (ACT's queue is empty by
                    # then) so the final two descriptor generations overlap
                    # instead of serializing on the sync sequencer.
                    ring = nc.scalar if b == NB - 2 else nc.sync
                    ring.dma_start(
                        out=out_t[:, s0[b] : s0[b + 1], :],
                        in_=o_sb[:, g0 : g0 + bs * _N].rearrange(
                            "p (s f) -> p s f", s=bs
                        ),
                    )
    nc.finalize()
    return nc


def _get_compiled():
    global _compiled
    if _compiled is None:
        _compiled = _build_nc()
    return _compiled


def run_on_hw(x: np.ndarray, w_l: np.ndarray, w_h: np.ndarray, trace: bool = False):
    """Returns ((LL, LH, HL, HH), exec_time_ns or None)."""
    from concourse.bass_utils import run_bass_kernel_spmd

    x = np.asarray(x, dtype=np.float32)
    W2 = _build_w2(np.asarray(w_l), np.asarray(w_h)).astype(np.float16)

    xf = x.reshape(-1, _N, _N)  # (768, 128, 128)
    nc = _get_compiled()
    in_maps = []
    for i in range(_NCORES):
        shard = xf[i * _S : (i + 1) * _S].transpose(1, 0, 2) * np.float32(_SCALE)
        in_maps.append(
            {
                "xh": np.ascontiguousarray(shard.astype(np.float16)),
                "w2": W2,
            }
        )
    res = run_bass_kernel_spmd(nc, in_maps, list(range(_NCORES)), trace=trace)

    quads = [[], [], [], []]  # LL, LH, HL, HH per-core chunks, each (S, 64, 64)
    for i in range(_NCORES):
        ot = res.results[i]["out_t"].astype(np.float32)  # (128, 96, 128) = [j(+64*qr), s, i(+64*qc)]
        quads[0].append(np.transpose(ot[0:64, :, 0:64], (1, 2, 0)))
        quads[1].append(np.transpose(ot[0:64, :, 64:128], (1, 2, 0)))
        quads[2].append(np.transpose(ot[64:128, :, 0:64], (1, 2, 0)))
        quads[3].append(np.transpose(ot[64:128, :, 64:128], (1, 2, 0)))

    B, C, H, W = x.shape
    out = tuple(
        np.ascontiguousarray(np.concatenate(q, axis=0)).reshape(B, C, H // 2, W // 2)
        for q in quads
    )
    return out, res.exec_time_ns


def kernel(x: np.ndarray, w_l: np.ndarray, w_h: np.ndarray):
    out, _ = run_on_hw(x, w_l, w_h, trace=False)
    return out


# revision 27
# speedup vs baseline: 1.0320x; 1.0320x over previous
"""2D DWT (db2, FFT-equivalent circular conv) as TensorE matmuls on 8 trn2 cores.

Math: for each (b,c) slice X (128x128), with F[k,j] = w[t] at k=(2j+2-t) mod 128
(the circular 4-tap filter + stride-2 decimation as a 128x64 matrix):
    LL = Fl^T X Fl,  LH = Fh^T X Fl,  HL = Fl^T X Fh,  HH = Fh^T X Fh.
With W2 = [Fl | Fh] (128x128):
    stage 1:  out1 = X^T @ W2 = [B_lT | B_hT]           (w on partitions)
    stage 2:  out2 = W2^T @ out1 = [[LL^T, LH^T], [HL^T, HH^T]]
out2 has partitions = j (W-direction output), free = i (H-direction output);
the final transpose of each 64x64 quadrant happens on the host at gather time.

Everything rides fp16 end to end: x, W2, the stage-1 intermediate, and the
DRAM output are all fp16 (PSUM accumulates in fp32).  Input quantization to
fp16 bounds the output error at ~1e-3 relative to max, far inside the 2e-2
gate, and it halves DMA traffic (the kernel is HBM-bound) while tripling
TensorE throughput vs an fp16 hi+lo split.  X is pre-scaled by 1024 on the
host so near-zero values stay out of fp16 subnormal range; the scale is
divided out in the stage-2 output copy (free affine on ACT / mul on DVE).

The whole per-core problem fits in SBUF (x/y/out are 24.6 KiB/partition
each), so x, the stage-1 intermediate y, and the staged output live in
single whole-shard tiles.  All DMA triggers ride the sync ring: inputs
back-to-back first, outputs strictly after, so no output trigger's
semaphore wait ever blocks an input and the ACT/DVE queues carry only
compute.  Work is software-pipelined per bank with stage 2 lagging
stage 1 by one bank on TensorE; banks are graduated (4-slice at both
ends, 8-slice in the middle) so the first drain chain -- and with it the
DVE and output streams -- starts ~2us sooner and the final drain tail is
short.  Each PSUM->SBUF drain is one wide instruction, and each stage
owns one engine (stage 1 -> ACT, stage 2 -> DVE) so neither engine's
in-order queue ever holds the other stage's work hostage to a semaphore.
Dummy matmuls on a memset tile warm the PE through the HAM half-clock
power ramp while the first input chunks are still in flight.

Sharding: 768 (b,c) slices split contiguously, 96 per core; pure data parallel.
Per-core input shards are transposed on the host to (h, s, w) so every DMA
reads multi-KB contiguous runs per partition.
"""

import numpy as np

_NCORES = 8
_S = 96          # slices per core
_N = 128
_SCALE = 1024.0  # fp16 subnormal guard; divided out in stage-2 copies

_compiled = None


def _build_w2(w_l: np.ndarray, w_h: np.ndarray) -> np.ndarray:
    W2 = np.zeros((_N, _N), dtype=np.float32)
    for col, w in ((0, w_l), (64, w_h)):
        w = np.asarray(w, dtype=np.float32).reshape(-1)
        L = w.shape[0]
        for j in range(_N // 2):
            for t in range(L):
                W2[(2 * j + L // 2 - t) % _N, col + j] += w[t]
    return W2


def _build_nc():
    import concourse.bacc as bacc
    import concourse.tile as tile
    import concourse.mybir as mybir

    f16 = mybir.dt.float16
    f32 = mybir.dt.float32
    nc = bacc.Bacc("TRN2", target_bir_lowering=False, debug=False)

    xh = nc.dram_tensor("xh", [_N, _S, _N], f16, kind="ExternalInput")  # (h, s, w)
    w2d = nc.dram_tensor("w2", [_N, _N], f16, kind="ExternalInput")
    out_t = nc.dram_tensor("out_t", [_N, _S, _N], f16, kind="ExternalOutput")
    inv = 1.0 / _SCALE

    # few input triggers: each costs ~650ns of sync-sequencer descriptor
    # generation, and the per-bank output triggers queue behind all of them
    in_chunks = [4, 4, 8, 16, 32, 32]
    assert sum(in_chunks) == _S
    # graduated superbanks: 4-slice banks at both ends shorten the first
    # drain chain (earlier DVE/output streams) and the final drain tail
    banks = [4, 4] + [8] * 10 + [4, 4]
    assert sum(banks) == _S
    NB = len(banks)
    s0 = [0]
    for bs in banks:
        s0.append(s0[-1] + bs)
    LAG = 1                                    # s2(b) issues after s1(b+LAG)
    copy_fn = mybir.ActivationFunctionType.Copy
    with tile.TileContext(nc) as tc:
        with (
            tc.tile_pool(name="singles", bufs=1) as singles,
            tc.tile_pool(name="ps1", bufs=2, space="PSUM") as ps1p,
            tc.tile_pool(name="ps2", bufs=2, space="PSUM") as ps2p,
        ):
            w2_sb = singles.tile([_N, _N], f16)
            # weights ride the scalar ring: its sequencer is free at body
            # start, so the tiny w2 descriptor generation runs concurrently
            # with chunk 0's on the sync ring instead of ahead of it
            nc.scalar.dma_start(out=w2_sb[:], in_=w2d[:])
            x_sb = singles.tile([_N, _S * _N], f16)
            y_sb = singles.tile([_N, _S * _N], f16)
            o_sb = singles.tile([_N, _S * _N], f16)

            # input stream: back-to-back on the sync ring
            c0 = 0
            for G in in_chunks:
                nc.sync.dma_start(
                    out=x_sb[:, c0 * _N : (c0 + G) * _N].rearrange(
                        "p (s w) -> p s w", s=G
                    ),
                    in_=xh[:, c0 : c0 + G, :],
                )
                c0 += G

            # warm the PE while the first input chunk is in flight: the HAM
            # power ramp holds the PE at half clock early on and releases it
            # as PE-busy time accumulates, so burn the DMA-wait window on
            # dummy matmuls (measured: trimming these slides the warm flip
            # later and costs more than the queue time they occupy).  The
            # operand is an on-chip memset tile, so the warm-up has no DMA
            # dependency and the PE stream starts the moment the body begins.
            warm_w = singles.tile([_N, _N], f16)
            nc.gpsimd.memset(warm_w[:], 0.0)
            warm = ps1p.tile([_N, 1024], f32, tag="ps1")
            for k in range(24):
                nc.tensor.matmul(
                    warm[:, :_N], lhsT=warm_w[:], rhs=warm_w[:],
                    start=(k == 0), stop=(k == 23),
                )

            for t in range(NB + LAG):
                if t < NB:
                    # stage 1, bank t -> PSUM, drained by one ACT copy
                    bs = banks[t]
                    ps1 = ps1p.tile([_N, 1024], f32, tag="ps1")
                    for k in range(bs):
                        s = s0[t] + k
                        nc.tensor.matmul(
                            ps1[:, k * _N : (k + 1) * _N],
                            lhsT=x_sb[:, s * _N : (s + 1) * _N],
                            rhs=w2_sb[:],
                            start=True,
                            stop=True,
                        )
                    # stage-1 drains always ride ACT: a single-purpose FIFO
                    # never holds a stage-2 copy hostage to a stage-1 sem
                    nc.scalar.copy(
                        out=y_sb[:, s0[t] * _N : s0[t + 1] * _N],
                        in_=ps1[:, : bs * _N],
                    )

                if t >= LAG:
                    # stage 2, bank b: 512-col matmuls into one PSUM tile,
                    # drained by a single DVE copy (the pacing engine)
                    b = t - LAG
                    bs = banks[b]
                    g0 = s0[b] * _N
                    ps2 = ps2p.tile([_N, 1024], f32, tag="ps2")
                    for h in range((bs * _N + 511) // 512):
                        gw = min(512, bs * _N - h * 512)
                        nc.tensor.matmul(
                            ps2[:, h * 512 : h * 512 + gw],
                            lhsT=w2_sb[:], rhs=y_sb[:, g0 + h * 512 : g0 + h * 512 + gw],
                            start=True, stop=True,
                        )
                    nc.vector.tensor_scalar_mul(
                        o_sb[:, g0 : g0 + bs * _N], ps2[:, : bs * _N], inv
                    )

                    # per-bank output DMA: the stream starts as soon as the
                    # very first bank's drain lands.  The second-to-last
                    # trigger rides the scalar ring (ACT's queue is empty by
                    # then) so the final two descriptor generations overlap
                    # instead of serializing on the sync sequencer.
                    ring = nc.scalar if b == NB - 2 else nc.sync
                    ring.dma_start(
                        out=out_t[:, s0[b] : s0[b + 1], :],
                        in_=o_sb[:, g0 : g0 + bs * _N].rearrange(
                            "p (s f) -> p s f", s=bs
                        ),
                    )
    nc.finalize()
    return nc


def _get_compiled():
    global _compiled
    if _compiled is None:
        _compiled = _build_nc()
    return _compiled


def run_on_hw(x: np.ndarray, w_l: np.ndarray, w_h: np.ndarray, trace: bool = False):
    """Returns ((LL, LH, HL, HH), exec_time_ns or None)."""
    from concourse.bass_utils import run_bass_kernel_spmd

    x = np.asarray(x, dtype=np.float32)
    W2 = _build_w2(np.asarray(w_l), np.asarray(w_h)).astype(np.float16)

    xf = x.reshape(-1, _N, _N)  # (768, 128, 128)
    nc = _get_compiled()
    in_maps = []
    for i in range(_NCORES):
        shard = xf[i * _S : (i + 1) * _S].transpose(1, 0, 2) * np.float32(_SCALE)
        in_maps.append(
            {
                "xh": np.ascontiguousarray(shard.astype(np.float16)),
                "w2": W2,
            }
        )
    res = run_bass_kernel_spmd(nc, in_maps, list(range(_NCORES)), trace=trace)

    quads = [[], [], [], []]  # LL, LH, HL, HH per-core chunks, each (S, 64, 64)
    for i in range(_NCORES):
        ot = res.results[i]["out_t"].astype(np.float32)  # (128, 96, 128) = [j(+64*qr), s, i(+64*qc)]
        quads[0].append(np.transpose(ot[0:64, :, 0:64], (1, 2, 0)))
        quads[1].append(np.transpose(ot[0:64, :, 64:128], (1, 2, 0)))
        quads[2].append(np.transpose(ot[64:128, :, 0:64], (1, 2, 0)))
        quads[3].append(np.transpose(ot[64:128, :, 64:128], (1, 2, 0)))

    B, C, H, W = x.shape
    out = tuple(
        np.ascontiguousarray(np.concatenate(q, axis=0)).reshape(B, C, H // 2, W // 2)
        for q in quads
    )
    return out, res.exec_time_ns


def kernel(x: np.ndarray, w_l: np.ndarray, w_h: np.ndarray):
    out, _ = run_on_hw(x, w_l, w_h, trace=False)
    return out


# revision 28
# speedup vs baseline: 1.0448x; 1.0124x over previous
"""2D DWT (db2, FFT-equivalent circular conv) as TensorE matmuls on 8 trn2 cores.

Math: for each (b,c) slice X (128x128), with F[k,j] = w[t] at k=(2j+2-t) mod 128
(the circular 4-tap filter + stride-2 decimation as a 128x64 matrix):
    LL = Fl^T X Fl,  LH = Fh^T X Fl,  HL = Fl^T X Fh,  HH = Fh^T X Fh.
With W2 = [Fl | Fh] (128x128):
    stage 1:  out1 = X^T @ W2 = [B_lT | B_hT]           (w on partitions)
    stage 2:  out2 = W2^T @ out1 = [[LL^T, LH^T], [HL^T, HH^T]]
out2 has partitions = j (W-direction output), free = i (H-direction output);
the final transpose of each 64x64 quadrant happens on the host at gather time.

Everything rides fp16 end to end: x, W2, the stage-1 intermediate, and the
DRAM output are all fp16 (PSUM accumulates in fp32).  Input quantization to
fp16 bounds the output error at ~1e-3 relative to max, far inside the 2e-2
gate, and it halves DMA traffic (the kernel is HBM-bound) while tripling
TensorE throughput vs an fp16 hi+lo split.  X is pre-scaled by 1024 on the
host so near-zero values stay out of fp16 subnormal range; the scale is
divided out in the stage-2 output copy (free affine on ACT / mul on DVE).

The whole per-core problem fits in SBUF (x/y/out are 24.6 KiB/partition
each), so x, the stage-1 intermediate y, and the staged output live in
single whole-shard tiles.  All DMA triggers ride the sync ring: inputs
back-to-back first, outputs strictly after, so no output trigger's
semaphore wait ever blocks an input and the ACT/DVE queues carry only
compute.  Work is software-pipelined per bank with stage 2 lagging
stage 1 by one bank on TensorE; banks are graduated (4-slice at both
ends, 8-slice in the middle) so the first drain chain -- and with it the
DVE and output streams -- starts ~2us sooner and the final drain tail is
short.  Each PSUM->SBUF drain is one wide instruction, and each stage
owns one engine (stage 1 -> ACT, stage 2 -> DVE) so neither engine's
in-order queue ever holds the other stage's work hostage to a semaphore.
Dummy matmuls on a memset tile warm the PE through the HAM half-clock
power ramp while the first input chunks are still in flight.

Sharding: 768 (b,c) slices split contiguously, 96 per core; pure data parallel.
Per-core input shards are transposed on the host to (h, s, w) so every DMA
reads multi-KB contiguous runs per partition.
"""

import numpy as np

_NCORES = 8
_S = 96          # slices per core
_N = 128
_SCALE = 1024.0  # fp16 subnormal guard; divided out in stage-2 copies

_compiled = None


def _build_w2(w_l: np.ndarray, w_h: np.ndarray) -> np.ndarray:
    W2 = np.zeros((_N, _N), dtype=np.float32)
    for col, w in ((0, w_l), (64, w_h)):
        w = np.asarray(w, dtype=np.float32).reshape(-1)
        L = w.shape[0]
        for j in range(_N // 2):
            for t in range(L):
                W2[(2 * j + L // 2 - t) % _N, col + j] += w[t]
    return W2


def _build_nc():
    import concourse.bacc as bacc
    import concourse.tile as tile
    import concourse.mybir as mybir

    f16 = mybir.dt.float16
    f32 = mybir.dt.float32
    nc = bacc.Bacc("TRN2", target_bir_lowering=False, debug=False)

    xh = nc.dram_tensor("xh", [_N, _S, _N], f16, kind="ExternalInput")  # (h, s, w)
    w2d = nc.dram_tensor("w2", [_N, _N], f16, kind="ExternalInput")
    out_t = nc.dram_tensor("out_t", [_N, _S, _N], f16, kind="ExternalOutput")
    inv = 1.0 / _SCALE

    # few input triggers: each costs ~650ns of sync-sequencer descriptor
    # generation, and the per-bank output triggers queue behind all of them
    in_chunks = [4, 4, 8, 16, 16, 16, 16, 16]
    assert sum(in_chunks) == _S
    # graduated superbanks: 4-slice banks at both ends shorten the first
    # drain chain (earlier DVE/output streams) and the final drain tail
    banks = [4, 4] + [8] * 10 + [4, 4]
    assert sum(banks) == _S
    NB = len(banks)
    s0 = [0]
    for bs in banks:
        s0.append(s0[-1] + bs)
    LAG = 1                                    # s2(b) issues after s1(b+LAG)
    copy_fn = mybir.ActivationFunctionType.Copy
    with tile.TileContext(nc) as tc:
        with (
            tc.tile_pool(name="singles", bufs=1) as singles,
            tc.tile_pool(name="ps1", bufs=2, space="PSUM") as ps1p,
            tc.tile_pool(name="ps2", bufs=2, space="PSUM") as ps2p,
        ):
            w2_sb = singles.tile([_N, _N], f16)
            # weights ride the scalar ring: its sequencer is free at body
            # start, so the tiny w2 descriptor generation runs concurrently
            # with chunk 0's on the sync ring instead of ahead of it
            nc.scalar.dma_start(out=w2_sb[:], in_=w2d[:])
            x_sb = singles.tile([_N, _S * _N], f16)
            y_sb = singles.tile([_N, _S * _N], f16)
            o_sb = singles.tile([_N, _S * _N], f16)

            # input stream: back-to-back on the sync ring
            c0 = 0
            for G in in_chunks:
                nc.sync.dma_start(
                    out=x_sb[:, c0 * _N : (c0 + G) * _N].rearrange(
                        "p (s w) -> p s w", s=G
                    ),
                    in_=xh[:, c0 : c0 + G, :],
                )
                c0 += G

            # warm the PE while the first input chunk is in flight: the HAM
            # power ramp holds the PE at half clock early on and releases it
            # as PE-busy time accumulates, so burn the DMA-wait window on
            # dummy matmuls (measured: trimming these slides the warm flip
            # later and costs more than the queue time they occupy).  The
            # operand is an on-chip memset tile, so the warm-up has no DMA
            # dependency and the PE stream starts the moment the body begins.
            warm_w = singles.tile([_N, _N], f16)
            nc.gpsimd.memset(warm_w[:], 0.0)
            warm = ps1p.tile([_N, 1024], f32, tag="ps1")
            for k in range(24):
                nc.tensor.matmul(
                    warm[:, :_N], lhsT=warm_w[:], rhs=warm_w[:],
                    start=(k == 0), stop=(k == 23),
                )

            for t in range(NB + LAG):
                if t < NB:
                    # stage 1, bank t -> PSUM, drained by one ACT copy
                    bs = banks[t]
                    ps1 = ps1p.tile([_N, 1024], f32, tag="ps1")
                    for k in range(bs):
                        s = s0[t] + k
                        nc.tensor.matmul(
                            ps1[:, k * _N : (k + 1) * _N],
                            lhsT=x_sb[:, s * _N : (s + 1) * _N],
                            rhs=w2_sb[:],
                            start=True,
                            stop=True,
                        )
                    # stage-1 drains always ride ACT: a single-purpose FIFO
                    # never holds a stage-2 copy hostage to a stage-1 sem
                    nc.scalar.copy(
                        out=y_sb[:, s0[t] * _N : s0[t + 1] * _N],
                        in_=ps1[:, : bs * _N],
                    )

                if t >= LAG:
                    # stage 2, bank b: 512-col matmuls into one PSUM tile,
                    # drained by a single DVE copy (the pacing engine)
                    b = t - LAG
                    bs = banks[b]
                    g0 = s0[b] * _N
                    ps2 = ps2p.tile([_N, 1024], f32, tag="ps2")
                    for h in range((bs * _N + 511) // 512):
                        gw = min(512, bs * _N - h * 512)
                        nc.tensor.matmul(
                            ps2[:, h * 512 : h * 512 + gw],
                            lhsT=w2_sb[:], rhs=y_sb[:, g0 + h * 512 : g0 + h * 512 + gw],
                            start=True, stop=True,
                        )
                    nc.vector.tensor_scalar_mul(
                        o_sb[:, g0 : g0 + bs * _N], ps2[:, : bs * _N], inv
                    )

                    # per-bank output DMA: the stream starts as soon as the
                    # very first bank's drain lands.  The second-to-last
                    # trigger rides the scalar ring (ACT's queue is empty by
                    # then) so the final two descriptor generations overlap
                    # instead of serializing on the sync sequencer.
                    ring = nc.scalar if b == NB - 2 else nc.sync
                    ring.dma_start(
                        out=out_t[:, s0[b] : s0[b + 1], :],
                        in_=o_sb[:, g0 : g0 + bs * _N].rearrange(
                            "p (s f) -> p s f", s=bs
                        ),
                    )
    nc.finalize()
    return nc


def _get_compiled():
    global _compiled
    if _compiled is None:
        _compiled = _build_nc()
    return _compiled


def run_on_hw(x: np.ndarray, w_l: np.ndarray, w_h: np.ndarray, trace: bool = False):
    """Returns ((LL, LH, HL, HH), exec_time_ns or None)."""
    from concourse.bass_utils import run_bass_kernel_spmd

    x = np.asarray(x, dtype=np.float32)
    W2 = _build_w2(np.asarray(w_l), np.asarray(w_h)).astype(np.float16)

    xf = x.reshape(-1, _N, _N)  # (768, 128, 128)
    nc = _get_compiled()
    in_maps = []
    for i in range(_NCORES):
        shard = xf[i * _S : (i + 1) * _S].transpose(1, 0, 2) * np.float32(_SCALE)
        in_maps.append(
            {
                "xh": np.ascontiguousarray(shard.astype(np.float16)),
                "w2": W2,
            }
        )
    res = run_bass_kernel_spmd(nc, in_maps, list(range(_NCORES)), trace=trace)

    quads = [[], [], [], []]  # LL, LH, HL, HH per-core chunks, each (S, 64, 64)
    for i in range(_NCORES):
        ot = res.results[i]["out_t"].astype(np.float32)  # (128, 96, 128) = [j(+64*qr), s, i(+64*qc)]
        quads[0].append(np.transpose(ot[0:64, :, 0:64], (1, 2, 0)))
        quads[1].append(np.transpose(ot[0:64, :, 64:128], (1, 2, 0)))
        quads[2].append(np.transpose(ot[64:128, :, 0:64], (1, 2, 0)))
        quads[3].append(np.transpose(ot[64:128, :, 64:128], (1, 2, 0)))

    B, C, H, W = x.shape
    out = tuple(
        np.ascontiguousarray(np.concatenate(q, axis=0)).reshape(B, C, H // 2, W // 2)
        for q in quads
    )
    return out, res.exec_time_ns


def kernel(x: np.ndarray, w_l: np.ndarray, w_h: np.ndarray):
    out, _ = run_on_hw(x, w_l, w_h, trace=False)
    return out


# revision 29
# speedup vs baseline: 1.0980x; 1.0509x over previous
"""2D DWT (db2, FFT-equivalent circular conv) as TensorE matmuls on 8 trn2 cores.

Math: for each (b,c) slice X (128x128), with F[k,j] = w[t] at k=(2j+2-t) mod 128
(the circular 4-tap filter + stride-2 decimation as a 128x64 matrix):
    LL = Fl^T X Fl,  LH = Fh^T X Fl,  HL = Fl^T X Fh,  HH = Fh^T X Fh.
With W2 = [Fl | Fh] (128x128):
    stage 1:  out1 = X^T @ W2 = [B_lT | B_hT]           (w on partitions)
    stage 2:  out2 = W2^T @ out1 = [[LL^T, LH^T], [HL^T, HH^T]]
out2 has partitions = j (W-direction output), free = i (H-direction output);
the final transpose of each 64x64 quadrant happens on the host at gather time.

Everything rides fp16 end to end: x, W2, the stage-1 intermediate, and the
DRAM output are all fp16 (PSUM accumulates in fp32).  Input quantization to
fp16 bounds the output error at ~1e-3 relative to max, far inside the 2e-2
gate, and it halves DMA traffic (the kernel is HBM-bound) while tripling
TensorE throughput vs an fp16 hi+lo split.  X is pre-scaled by 1024 on the
host so near-zero values stay out of fp16 subnormal range; the scale is
divided out in the stage-2 output copy (free affine on ACT / mul on DVE).

The whole per-core problem fits in SBUF (x/y/out are 24.6 KiB/partition
each), so x, the stage-1 intermediate y, and the staged output live in
single whole-shard tiles.  All DMA triggers ride the sync ring: inputs
back-to-back first, outputs strictly after, so no output trigger's
semaphore wait ever blocks an input and the ACT/DVE queues carry only
compute.  Work is software-pipelined per bank with stage 2 lagging
stage 1 by one bank on TensorE; banks are graduated (4-slice at both
ends, 8-slice in the middle) so the first drain chain -- and with it the
DVE and output streams -- starts ~2us sooner and the final drain tail is
short.  Each PSUM->SBUF drain is one wide instruction, and each stage
owns one engine (stage 1 -> ACT, stage 2 -> DVE) so neither engine's
in-order queue ever holds the other stage's work hostage to a semaphore.
Dummy matmuls on a memset tile warm the PE through the HAM half-clock
power ramp while the first input chunks are still in flight.

Sharding: 768 (b,c) slices split contiguously, 96 per core; pure data parallel.
Per-core input shards are transposed on the host to (h, s, w) so every DMA
reads multi-KB contiguous runs per partition.
"""

import numpy as np

_NCORES = 8
_S = 96          # slices per core
_N = 128
_SCALE = 1024.0  # fp16 subnormal guard; divided out in stage-2 copies

_compiled = None


def _build_w2(w_l: np.ndarray, w_h: np.ndarray) -> np.ndarray:
    W2 = np.zeros((_N, _N), dtype=np.float32)
    for col, w in ((0, w_l), (64, w_h)):
        w = np.asarray(w, dtype=np.float32).reshape(-1)
        L = w.shape[0]
        for j in range(_N // 2):
            for t in range(L):
                W2[(2 * j + L // 2 - t) % _N, col + j] += w[t]
    return W2


def _build_nc():
    import concourse.bacc as bacc
    import concourse.tile as tile
    import concourse.mybir as mybir

    f16 = mybir.dt.float16
    f32 = mybir.dt.float32
    nc = bacc.Bacc("TRN2", target_bir_lowering=False, debug=False)

    xh = nc.dram_tensor("xh", [_N, _S, _N], f16, kind="ExternalInput")  # (h, s, w)
    w2d = nc.dram_tensor("w2", [_N, _N], f16, kind="ExternalInput")
    out_t = nc.dram_tensor("out_t", [_N, _S, _N], f16, kind="ExternalOutput")
    inv = 1.0 / _SCALE

    # input chunking balances three costs: each trigger burns ~650ns of
    # sync-sequencer descriptor generation (and the per-bank output triggers
    # queue behind all of them), coarser chunks mean coarser completion
    # semaphores that stall mid-stream stage-1 matmuls, and the first chunks
    # gate how soon compute starts.  Fine at the head, 16-slice after.
    in_chunks = [4, 4, 8, 16, 16, 16, 16, 16]
    assert sum(in_chunks) == _S
    # graduated superbanks: 4-slice banks at both ends shorten the first
    # drain chain (earlier DVE/output streams) and the final drain tail
    banks = [4, 4] + [8] * 10 + [4, 4]
    assert sum(banks) == _S
    NB = len(banks)
    s0 = [0]
    for bs in banks:
        s0.append(s0[-1] + bs)
    LAG = 1                                    # s2(b) issues after s1(b+LAG)
    copy_fn = mybir.ActivationFunctionType.Copy
    with tile.TileContext(nc) as tc:
        with (
            tc.tile_pool(name="singles", bufs=1) as singles,
            tc.tile_pool(name="ps1", bufs=2, space="PSUM") as ps1p,
            tc.tile_pool(name="ps2", bufs=2, space="PSUM") as ps2p,
        ):
            w2_sb = singles.tile([_N, _N], f16)
            # weights ride the scalar ring: its sequencer is free at body
            # start, so the tiny w2 descriptor generation runs concurrently
            # with chunk 0's on the sync ring instead of ahead of it
            nc.scalar.dma_start(out=w2_sb[:], in_=w2d[:])
            x_sb = singles.tile([_N, _S * _N], f16)
            y_sb = singles.tile([_N, _S * _N], f16)
            o_sb = singles.tile([_N, _S * _N], f16)

            # input stream: back-to-back on the sync ring
            c0 = 0
            for G in in_chunks:
                nc.sync.dma_start(
                    out=x_sb[:, c0 * _N : (c0 + G) * _N].rearrange(
                        "p (s w) -> p s w", s=G
                    ),
                    in_=xh[:, c0 : c0 + G, :],
                )
                c0 += G

            # warm the PE while the first input chunk is in flight: the HAM
            # power ramp holds the PE at half clock early on and releases it
            # as PE-busy time accumulates, so burn the DMA-wait window on
            # dummy matmuls (measured: trimming these slides the warm flip
            # later and costs more than the queue time they occupy).  The
            # operand is an on-chip memset tile, so the warm-up has no DMA
            # dependency and the PE stream starts the moment the body begins.
            warm_w = singles.tile([_N, _N], f16)
            nc.gpsimd.memset(warm_w[:], 0.0)
            warm = ps1p.tile([_N, 1024], f32, tag="ps1")
            for k in range(24):
                nc.tensor.matmul(
                    warm[:, :_N], lhsT=warm_w[:], rhs=warm_w[:],
                    start=(k == 0), stop=(k == 23),
                )

            for t in range(NB + LAG):
                if t < NB:
                    # stage 1, bank t -> PSUM, drained by one ACT copy
                    bs = banks[t]
                    ps1 = ps1p.tile([_N, 1024], f32, tag="ps1")
                    for k in range(bs):
                        s = s0[t] + k
                        nc.tensor.matmul(
                            ps1[:, k * _N : (k + 1) * _N],
                            lhsT=x_sb[:, s * _N : (s + 1) * _N],
                            rhs=w2_sb[:],
                            start=True,
                            stop=True,
                        )
                    # stage-1 drains always ride ACT: a single-purpose FIFO
                    # never holds a stage-2 copy hostage to a stage-1 sem
                    nc.scalar.copy(
                        out=y_sb[:, s0[t] * _N : s0[t + 1] * _N],
                        in_=ps1[:, : bs * _N],
                    )

                if t >= LAG:
                    # stage 2, bank b: 512-col matmuls into one PSUM tile,
                    # drained by a single DVE copy (the pacing engine)
                    b = t - LAG
                    bs = banks[b]
                    g0 = s0[b] * _N
                    ps2 = ps2p.tile([_N, 1024], f32, tag="ps2")
                    for h in range((bs * _N + 511) // 512):
                        gw = min(512, bs * _N - h * 512)
                        nc.tensor.matmul(
                            ps2[:, h * 512 : h * 512 + gw],
                            lhsT=w2_sb[:], rhs=y_sb[:, g0 + h * 512 : g0 + h * 512 + gw],
                            start=True, stop=True,
                        )
                    nc.vector.tensor_scalar_mul(
                        o_sb[:, g0 : g0 + bs * _N], ps2[:, : bs * _N], inv
                    )

                    # per-bank output DMA: the stream starts as soon as the
                    # very first bank's drain lands.  The second-to-last
                    # trigger rides the scalar ring (ACT's queue is empty by
                    # then) so the final two descriptor generations overlap
                    # instead of serializing on the sync sequencer.
                    ring = nc.scalar if b == NB - 2 else nc.sync
                    ring.dma_start(
                        out=out_t[:, s0[b] : s0[b + 1], :],
                        in_=o_sb[:, g0 : g0 + bs * _N].rearrange(
                            "p (s f) -> p s f", s=bs
                        ),
                    )
    nc.finalize()
    return nc


def _get_compiled():
    global _compiled
    if _compiled is None:
        _compiled = _build_nc()
    return _compiled


def run_on_hw(x: np.ndarray, w_l: np.ndarray, w_h: np.ndarray, trace: bool = False):
    """Returns ((LL, LH, HL, HH), exec_time_ns or None)."""
    from concourse.bass_utils import run_bass_kernel_spmd

    x = np.asarray(x, dtype=np.float32)
    W2 = _build_w2(np.asarray(w_l), np.asarray(w_h)).astype(np.float16)

    xf = x.reshape(-1, _N, _N)  # (768, 128, 128)
    nc = _get_compiled()
    in_maps = []
    for i in range(_NCORES):
        shard = xf[i * _S : (i + 1) * _S].transpose(1, 0, 2) * np.float32(_SCALE)
        in_maps.append(
            {
                "xh": np.ascontiguousarray(shard.astype(np.float16)),
                "w2": W2,
            }
        )
    res = run_bass_kernel_spmd(nc, in_maps, list(range(_NCORES)), trace=trace)

    quads = [[], [], [], []]  # LL, LH, HL, HH per-core chunks, each (S, 64, 64)
    for i in range(_NCORES):
        ot = res.results[i]["out_t"].astype(np.float32)  # (128, 96, 128) = [j(+64*qr), s, i(+64*qc)]
        quads[0].append(np.transpose(ot[0:64, :, 0:64], (1, 2, 0)))
        quads[1].append(np.transpose(ot[0:64, :, 64:128], (1, 2, 0)))
        quads[2].append(np.transpose(ot[64:128, :, 0:64], (1, 2, 0)))
        quads[3].append(np.transpose(ot[64:128, :, 64:128], (1, 2, 0)))

    B, C, H, W = x.shape
    out = tuple(
        np.ascontiguousarray(np.concatenate(q, axis=0)).reshape(B, C, H // 2, W // 2)
        for q in quads
    )
    return out, res.exec_time_ns


def kernel(x: np.ndarray, w_l: np.ndarray, w_h: np.ndarray):
    out, _ = run_on_hw(x, w_l, w_h, trace=False)
    return out
